# revision 10
# baseline (speedup 1.0000x reference)
"""CTC loss (Keras ctc_batch_cost semantics) on 8 Trainium2 NeuronCores.

Strategy: pure data parallelism — batch B=1024 sharded 128/core (batch =
SBUF partition dim). Host does index preparation only (extended-label
gather of y_pred, skip masks, reachability mask, seam/readout fold); each
core runs the CTC lattice sum in the linear-probability domain in bf16,
split into a forward DP (t=0..100) on the Vector engine and a backward
suffix DP in gamma form (G_t = p_t * B_t, t=255..101) on the Pool engine —
two independent serial chains, one per engine, running concurrently. The
seam sits at TM=100 so the slower-per-step DVE chain is shorter and both
engines finish together. Pool only supports TensorTensor, so every 8 steps
each chain is renormalized to 2^110 with scales computed on DVE (its own
max free via tensor_tensor_reduce; the Pool chain's via a DVE reduce) and
applied on-chain (DVE: fused scalar_tensor_tensor; Pool: an extra
tensor_tensor against a DVE-broadcast scale row). The chains meet in a
one-time log-domain seam (per-lane exact bit logs + logsumexp — lane
products span ~2^-175 and would underflow linear f32):
ll = m + ln Σ_s exp(lnA+lnB−m) + ln2·(Σ exponent shifts − 220).
No collectives; host concatenates the per-core [128,1] outputs.
"""

import numpy as np
import ml_dtypes

import concourse.bacc as bacc
import concourse.mybir as mybir
import concourse.tile as tile
from concourse.bass_utils import run_bass_kernel_spmd

B, T, C, U = 1024, 256, 100, 48
S = 2 * U + 1          # 97 extended-label positions
SP = 104               # S padded; bf16 row = 208B
BLANK = C - 1
EPS = 1e-7
NCORES = 8
BS = B // NCORES       # 128 samples per core = SBUF partition dim
CH = 16                # time steps per DMA chunk
TM = 100               # seam: fwd covers t=1..TM (DVE), bwd t=254..TM (Pool)
TB = T - 2 - TM        # 154 bwd steps
THF = TM + 1           # pf slots (slot 0 seeds alpha_0)
THB = TB + 1           # pb slots (slot 0 = sel*p_255)
RENORM = 8
RT_LOG2 = 110          # renorm target 2^110 (also the host init scale)
EV_F = [i for i in range(2, TM) if i % RENORM == 0]      # 12 events
EV_B = [i for i in range(2, TB) if i % RENORM == 0]      # 19 events
NEV_F = len(EV_F)
NEV_B = len(EV_B)
NLG = NEV_F + NEV_B
F32 = mybir.dt.float32
BF16 = mybir.dt.bfloat16
I32 = mybir.dt.int32
ALU = mybir.AluOpType
AXX = mybir.AxisListType.X
ACTF = mybir.ActivationFunctionType
LN2 = float(np.log(2.0))
TINY = 1e-38
DEAD = -1000.0


def _emit(nc, tc, pf_d, pb_d, skip_d, skip2_d, out_d):
    v = nc.vector
    g = nc.gpsimd
    with tc.tile_pool(name="pchunks", bufs=1) as ppool, tc.tile_pool(
        name="work", bufs=1
    ) as wp:
        skip_t = wp.tile([BS, SP], BF16, name="skip_t")
        nc.sync.dma_start(out=skip_t[:], in_=skip_d)
        skip2_t = wp.tile([BS, SP], BF16, name="skip2_t")
        nc.sync.dma_start(out=skip2_t[:], in_=skip2_d)
        pfs = []
        pbs = []
        nchf = (THF + CH - 1) // CH
        nchb = (THB + CH - 1) // CH
        for ci in range(max(nchf, nchb)):
            if ci < nchf:
                n = min(CH, THF - ci * CH)
                pf = ppool.tile([BS, n * SP], BF16, name=f"pf{ci}")
                nc.sync.dma_start(out=pf[:], in_=pf_d[:, ci * CH : ci * CH + n, :])
                pfs.append(pf)
            if ci < nchb:
                n = min(CH, THB - ci * CH)
                pb = ppool.tile([BS, n * SP], BF16, name=f"pb{ci}")
                nc.sync.dma_start(out=pb[:], in_=pb_d[:, ci * CH : ci * CH + n, :])
                pbs.append(pb)

        # forward state: 2 permanently-zero cols in FRONT (s-1/s-2 reads).
        fA = wp.tile([BS, SP + 2], BF16, name="fA")
        fB = wp.tile([BS, SP + 2], BF16, name="fB")
        # backward gamma state: cols S..SP-1 stay zero (s+1/s+2 reads).
        bA = wp.tile([BS, SP], BF16, name="bA")
        bB = wp.tile([BS, SP], BF16, name="bB")
        wf = wp.tile([BS, S], BF16, name="wf")
        vf = wp.tile([BS, S], BF16, name="vf")
        uf = wp.tile([BS, S], BF16, name="uf")
        wb = wp.tile([BS, S], BF16, name="wb")
        vb = wp.tile([BS, S], BF16, name="vb")
        ub = wp.tile([BS, S], BF16, name="ub")
        ub2 = wp.tile([BS, S], BF16, name="ub2")
        mx2 = wp.tile([BS, 2], F32, name="mx2")
        k2 = wp.tile([BS, 2], I32, name="k2")
        sc2 = wp.tile([BS, 2], I32, name="sc2")
        ones = wp.tile([BS, S], BF16, name="ones")
        bscw = wp.tile([BS, S], BF16, name="bscw")
        lgi = wp.tile([BS, NLG], I32, name="lgi")
        lgall = wp.tile([BS, NLG], F32, name="lgall")
        d_in = wp.tile([BS, 1], F32, name="d_in")
        d_out = wp.tile([BS, 1], F32, name="d_out")

        v.memset(fA[:], 0.0)
        v.memset(fB[:], 0.0)
        v.memset(bA[:], 0.0)
        v.memset(bB[:], 0.0)
        v.memset(lgi[:], 0)
        v.memset(ones[:], 1.0)
        # preload ACT Ln/Exp tables during the DMA wait (they're used only
        # in the seam; a cold table load there would serialize ~2.6us).
        v.memset(d_in[:], 1.0)
        nc.scalar.activation(out=d_out[:], in_=d_in[:], func=ACTF.Ln)
        nc.scalar.activation(out=d_out[:], in_=d_in[:], func=ACTF.Exp, bias=0.0, scale=1.0)
        # alpha_0: s=0 (blank) and s=1 (first label); host scaled by 2^110.
        v.tensor_copy(out=fA[:, 2:4], in_=pfs[0][:, 0:2])
        # G_255 = sel * p_255 * 2^110, host-baked into pb slot 0.
        v.tensor_copy(out=bA[:, 0:SP], in_=pbs[0][:, 0:SP])

        # col pairs of (k_fwd, k_bwd) while both chains run, then solo bwd.
        lg_pair = {i: 2 * e for e, i in enumerate(EV_F)}
        lg_solo = {i: 2 * NEV_F + e for e, i in enumerate(EV_B[NEV_F:])}

        def scalar_chain(mxi_ap, k_ap, sc_ap, lg_ap):
            """k = clamp(e(max)-110, -126) into lgi; sc = bits of 2^-k."""
            v.tensor_scalar(out=k_ap, in0=mxi_ap, scalar1=23, scalar2=None, op0=ALU.logical_shift_right)
            v.tensor_scalar(out=lg_ap, in0=k_ap, scalar1=127 + RT_LOG2, scalar2=-126, op0=ALU.subtract, op1=ALU.max)
            v.tensor_scalar(out=sc_ap, in0=lg_ap, scalar1=-1, scalar2=127, op0=ALU.mult, op1=ALU.add)
            v.tensor_scalar(out=sc_ap, in0=sc_ap, scalar1=23, scalar2=None, op0=ALU.logical_shift_left)

        for i in range(1, TB + 1):
            fon = i <= TM
            ev_f = i in EV_F
            ev_b = i in EV_B
            ap = i % RENORM == 1 and i > 1

            if fon:
                ci, off = i // CH, (i % CH) * SP
                pt = pfs[ci][:, off : off + S]
                fcur, fnxt = (fA, fB) if i % 2 == 1 else (fB, fA)
                # ---- forward step on DVE ----
                v.tensor_tensor(out=wf[:], in0=fcur[:, 2 : 2 + S], in1=fcur[:, 1 : 1 + S], op=ALU.add)
                v.tensor_tensor(out=vf[:], in0=fcur[:, 0:S], in1=skip_t[:, 0:S], op=ALU.mult)
                v.tensor_tensor(out=uf[:], in0=wf[:], in1=vf[:], op=ALU.add)
                if ev_f:
                    # tensor_tensor_reduce faults on HW; plain TT + reduce.
                    v.tensor_tensor(out=fnxt[:, 2 : 2 + S], in0=uf[:], in1=pt, op=ALU.mult)
                    v.tensor_reduce(out=mx2[:, 0:1], in_=fnxt[:, 2 : 2 + S], axis=AXX, op=ALU.max)
                elif ap:
                    v.scalar_tensor_tensor(
                        out=fnxt[:, 2 : 2 + S], in0=uf[:], scalar=sc2[:].bitcast(F32)[:, 0:1], in1=pt,
                        op0=ALU.mult, op1=ALU.mult,
                    )
                else:
                    v.tensor_tensor(out=fnxt[:, 2 : 2 + S], in0=uf[:], in1=pt, op=ALU.mult)

            # ---- backward gamma step on Pool (TensorTensor only) ----
            ci, off = i // CH, (i % CH) * SP
            qt = pbs[ci][:, off : off + S]
            bcur, bnxt = (bA, bB) if i % 2 == 1 else (bB, bA)
            g.tensor_tensor(out=wb[:], in0=bcur[:, 0:S], in1=bcur[:, 1 : 1 + S], op=ALU.add)
            g.tensor_tensor(out=vb[:], in0=bcur[:, 2 : 2 + S], in1=skip2_t[:, 0:S], op=ALU.mult)
            g.tensor_tensor(out=ub[:], in0=wb[:], in1=vb[:], op=ALU.add)
            if ap:
                g.tensor_tensor(out=ub2[:], in0=ub[:], in1=bscw[:], op=ALU.mult)
                g.tensor_tensor(out=bnxt[:, 0:S], in0=ub2[:], in1=qt, op=ALU.mult)
            else:
                g.tensor_tensor(out=bnxt[:, 0:S], in0=ub[:], in1=qt, op=ALU.mult)

            if ev_b:
                # Pool chain's max must come from DVE (Pool has no X reduce).
                v.tensor_reduce(out=mx2[:, 1:2], in_=bnxt[:, 0:S], axis=AXX, op=ALU.max)
                if ev_f:
                    c = lg_pair[i]
                    scalar_chain(mx2[:].bitcast(I32), k2[:], sc2[:], lgi[:, c : c + 2])
                else:
                    c = lg_solo[i]
                    scalar_chain(mx2[:].bitcast(I32)[:, 1:2], k2[:, 1:2], sc2[:, 1:2], lgi[:, c : c + 1])
                # broadcast the bwd scale to a full row for Pool's TT apply
                v.tensor_scalar(out=bscw[:], in0=ones[:], scalar1=sc2[:].bitcast(F32)[:, 1:2], scalar2=None, op0=ALU.mult)

        # TM=100 even and TB=154 even => finals live in fA / bA.
        ffin = fA[:, 2 : 2 + S]
        gfin = bA

        # ---- seam on Pool: B_TM = G + G^(+1) + skip2*G^(+2) ----
        bw2 = wp.tile([BS, S], BF16, name="bw2")
        bv2 = wp.tile([BS, S], BF16, name="bv2")
        bfin = wp.tile([BS, S], BF16, name="bfin")
        g.tensor_tensor(out=bw2[:], in0=gfin[:, 0:S], in1=gfin[:, 1 : 1 + S], op=ALU.add)
        g.tensor_tensor(out=bv2[:], in0=gfin[:, 2 : 2 + S], in1=skip2_t[:, 0:S], op=ALU.mult)
        g.tensor_tensor(out=bfin[:], in0=bw2[:], in1=bv2[:], op=ALU.add)

        # ---- log-domain seam on DVE/ACT ----
        ff32 = wp.tile([BS, S], F32, name="ff32")
        bf32 = wp.tile([BS, S], F32, name="bf32")
        as_t = wp.tile([BS, S], F32, name="as_t")
        ei_t = wp.tile([BS, S], I32, name="ei_t")
        mi_t = wp.tile([BS, S], I32, name="mi_t")
        ef_t = wp.tile([BS, S], F32, name="ef_t")
        lm_t = wp.tile([BS, S], F32, name="lm_t")
        la_t = wp.tile([BS, S], F32, name="la_t")
        da_t = wp.tile([BS, S], F32, name="da_t")
        la2 = wp.tile([BS, S], F32, name="la2")
        lb2 = wp.tile([BS, S], F32, name="lb2")
        lam = wp.tile([BS, S], F32, name="lam")
        m_t = wp.tile([BS, 1], F32, name="m_t")
        nm_t = wp.tile([BS, 1], F32, name="nm_t")
        e_t = wp.tile([BS, S], F32, name="e_t")
        z_t = wp.tile([BS, 1], F32, name="z_t")
        lnz_t = wp.tile([BS, 1], F32, name="lnz_t")
        racc = wp.tile([BS, 1], F32, name="racc")
        acc1 = wp.tile([BS, 1], F32, name="acc1")
        acc2 = wp.tile([BS, 1], F32, name="acc2")
        outt = wp.tile([BS, 1], F32, name="outt")

        def side_log(x32, out_lam):
            # exact log of f32 x: split exponent/mantissa so the Ln LUT
            # only sees [1,2); zero lanes forced to DEAD.
            # biased exponent; the uniform -127*ln2 per side is folded into
            # the final constant (racc) instead of a per-lane subtract.
            v.tensor_scalar(out=as_t[:], in0=x32[:], scalar1=1.0, scalar2=TINY, op0=ALU.mult, op1=ALU.add)
            ai = as_t[:].bitcast(I32)
            v.tensor_scalar(out=ei_t[:], in0=ai, scalar1=23, scalar2=None, op0=ALU.logical_shift_right)
            v.tensor_scalar(out=mi_t[:], in0=ai, scalar1=0x007FFFFF, scalar2=0x3F800000, op0=ALU.bitwise_and, op1=ALU.bitwise_or)
            v.tensor_copy(out=ef_t[:], in_=ei_t[:])
            nc.scalar.activation(out=lm_t[:], in_=mi_t[:].bitcast(F32), func=ACTF.Ln)
            v.scalar_tensor_tensor(out=la_t[:], in0=ef_t[:], scalar=LN2, in1=lm_t[:], op0=ALU.mult, op1=ALU.add)
            v.tensor_scalar(out=da_t[:], in0=x32[:], scalar1=0.0, scalar2=None, op0=ALU.is_equal)
            v.scalar_tensor_tensor(out=out_lam[:], in0=da_t[:], scalar=DEAD, in1=la_t[:], op0=ALU.mult, op1=ALU.add)

        v.tensor_copy(out=bf32[:], in_=bfin[:])
        side_log(bf32, lb2)
        v.tensor_copy(out=ff32[:], in_=ffin)
        side_log(ff32, la2)
        v.tensor_tensor(out=lam[:], in0=la2[:], in1=lb2[:], op=ALU.add)
        v.tensor_reduce(out=m_t[:], in_=lam[:], axis=AXX, op=ALU.max)
        v.tensor_scalar(out=nm_t[:], in0=m_t[:], scalar1=-1.0, scalar2=None, op0=ALU.mult)
        nc.scalar.activation(out=e_t[:], in_=lam[:], func=ACTF.Exp, bias=nm_t[:, 0:1], scale=1.0)
        v.tensor_reduce(out=z_t[:], in_=e_t[:], axis=AXX, op=ALU.add)
        nc.scalar.activation(out=lnz_t[:], in_=z_t[:], func=ACTF.Ln)
        v.tensor_copy(out=lgall[:], in_=lgi[:])
        v.tensor_reduce(out=racc[:], in_=lgall[:], axis=AXX, op=ALU.add)
        # constants: -2*RT for the host 2^110 injections, -2*127 for the two
        # side_log biased exponents.
        v.tensor_scalar(out=racc[:], in0=racc[:], scalar1=float(-2 * RT_LOG2 - 254), scalar2=None, op0=ALU.add)
        v.scalar_tensor_tensor(out=acc1[:], in0=racc[:], scalar=LN2, in1=m_t[:], op0=ALU.mult, op1=ALU.add)
        v.tensor_tensor(out=acc2[:], in0=acc1[:], in1=lnz_t[:], op=ALU.add)
        nc.scalar.mul(out=outt[:], in_=acc2[:], mul=-1.0)
        nc.sync.dma_start(out=out_d, in_=outt[:])


def _build_program():
    nc = bacc.Bacc("TRN2", target_bir_lowering=False, debug=False)
    pf_d = nc.dram_tensor("pf", [BS, THF, SP], BF16, kind="ExternalInput").ap()
    pb_d = nc.dram_tensor("pb", [BS, THB, SP], BF16, kind="ExternalInput").ap()
    skip_d = nc.dram_tensor("skip", [BS, SP], BF16, kind="ExternalInput").ap()
    skip2_d = nc.dram_tensor("skip2", [BS, SP], BF16, kind="ExternalInput").ap()
    out_d = nc.dram_tensor("out", [BS, 1], F32, kind="ExternalOutput").ap()
    with tile.TileContext(nc) as tc:
        _emit(nc, tc, pf_d, pb_d, skip_d, skip2_d, out_d)
    nc.compile()
    return nc


_NC = None


def _get_nc():
    global _NC
    if _NC is None:
        _NC = _build_program()
    return _NC


def _prep_in_maps(y_pred, y_true, label_length):
    ext = np.full((B, S), BLANK, np.int32)
    ext[:, 1::2] = y_true.astype(np.int32)
    prev2 = np.concatenate([np.full((B, 2), BLANK, np.int32), ext[:, :-2]], axis=1)
    skip = ((ext != BLANK) & (ext != prev2)).astype(np.float32)
    skip2 = np.concatenate([skip[:, 2:], np.zeros((B, 2), np.float32)], axis=1)
    P = np.take_along_axis(
        np.ascontiguousarray(y_pred, dtype=np.float32), ext[:, None, :], axis=2
    )
    P += np.float32(EPS)
    L = label_length.reshape(B).astype(np.int64)
    i2 = np.clip(2 * L, 0, S - 1)
    i1 = np.maximum(i2 - 1, 0)
    # reachability mask: position s at time t is dead if it cannot reach i1
    # by t=T-1 (max +2 per step). Folded into P at zero device cost.
    s_idx = np.arange(S)[None, None, :]
    t_idx = np.arange(T)[None, :, None]
    alive = (s_idx + 2 * (T - 1 - t_idx)) >= i1[:, None, None]
    P *= alive.astype(np.float32)
    # fwd chain eats t=0..TM in order; slot 0 only seeds alpha_0 (cols 0:2),
    # host-scaled to 2^110. bwd chain eats t=255..TM+1: slot j = p_{255-j};
    # slot 0 pre-multiplied by the readout selector and scaled 2^110.
    sel = np.zeros((B, S), np.float32)
    sel[np.arange(B), i1] = 1.0
    sel[np.arange(B), i2] = 1.0
    SCALE = np.float32(2.0**RT_LOG2)
    pf = np.zeros((B, THF, SP), np.float32)
    pf[:, :, :S] = P[:, :THF, :]
    pf[:, 0, :] *= SCALE
    pb = np.zeros((B, THB, SP), np.float32)
    pb[:, :, :S] = P[:, T - 1 : T - 1 - THB : -1, :]
    pb[:, 0, :S] *= sel * SCALE
    pf = pf.astype(ml_dtypes.bfloat16)
    pb = pb.astype(ml_dtypes.bfloat16)
    skipp = np.zeros((B, SP), np.float32)
    skipp[:, :S] = skip
    skip2p = np.zeros((B, SP), np.float32)
    skip2p[:, :S] = skip2
    skipp = skipp.astype(ml_dtypes.bfloat16)
    skip2p = skip2p.astype(ml_dtypes.bfloat16)
    in_maps = []
    for c in range(NCORES):
        sl = slice(c * BS, (c + 1) * BS)
        in_maps.append(
            {
                "pf": np.ascontiguousarray(pf[sl]),
                "pb": np.ascontiguousarray(pb[sl]),
                "skip": np.ascontiguousarray(skipp[sl]),
                "skip2": np.ascontiguousarray(skip2p[sl]),
            }
        )
    return in_maps


def _run_device(in_maps, **kwargs):
    nc = _get_nc()
    return run_bass_kernel_spmd(nc, in_maps, core_ids=list(range(NCORES)), **kwargs)


def _ctc_numpy(y_pred, y_true, input_length, label_length):
    """Generality safety net (log domain, mirrors the reference exactly)."""
    b, t_max, c = y_pred.shape
    u = y_true.shape[1]
    s = 2 * u + 1
    blank = c - 1
    neg = np.float32(-1e30)
    logp = np.log(y_pred.astype(np.float32) + np.float32(EPS))
    ext = np.full((b, s), blank, np.int32)
    ext[:, 1::2] = y_true.astype(np.int32)
    prev2 = np.concatenate([np.full((b, 2), blank, np.int32), ext[:, :-2]], axis=1)
    can_skip = (ext != blank) & (ext != prev2)
    lp_ext = np.take_along_axis(logp, ext[:, None, :], axis=2)
    alpha = np.full((b, s), neg, np.float32)
    alpha[:, 0] = lp_ext[:, 0, 0]
    alpha[:, 1] = lp_ext[:, 0, 1]
    inp_len = input_length.reshape(b)

    def lse(stack):
        m = np.max(stack, axis=0)
        return m + np.log(np.sum(np.exp(stack - m), axis=0))

    for t in range(1, t_max):
        a1 = np.concatenate([np.full((b, 1), neg, np.float32), alpha[:, :-1]], axis=1)
        a2 = np.concatenate([np.full((b, 2), neg, np.float32), alpha[:, :-2]], axis=1)
        a2 = np.where(can_skip, a2, neg)
        new = lse(np.stack([alpha, a1, a2], 0)).astype(np.float32) + lp_ext[:, t, :]
        alpha = np.where((t < inp_len)[:, None], new, alpha)
    L = label_length.reshape(b).astype(np.int64)
    i2 = np.clip(2 * L, 0, s - 1)
    i1 = np.maximum(i2 - 1, 0)
    a_last = np.stack([alpha[np.arange(b), i1], alpha[np.arange(b), i2]], axis=1)
    ll = np.where(L > 0, lse(a_last.T).astype(np.float32), alpha[:, 0])
    return (-ll[:, None]).astype(np.float32)


def kernel(y_pred, y_true, input_length, label_length):
    y_pred = np.asarray(y_pred)
    y_true = np.asarray(y_true)
    input_length = np.asarray(input_length)
    label_length = np.asarray(label_length)
    if y_pred.shape != (B, T, C) or y_true.shape != (B, U) or not np.all(
        input_length.reshape(-1) == T
    ) or np.any(label_length.reshape(-1) <= 0):
        return _ctc_numpy(y_pred, y_true, input_length, label_length)
    in_maps = _prep_in_maps(y_pred, y_true, label_length)
    res = _run_device(in_maps)
    out = np.concatenate([r["out"] for r in res.results], axis=0)
    return np.ascontiguousarray(out, dtype=np.float32)


# revision 12
# speedup vs baseline: 1.7649x; 1.7649x over previous
"""CTC loss (Keras ctc_batch_cost semantics) on 8 Trainium2 NeuronCores.

Strategy: pure data parallelism — batch B=1024 sharded 128/core (batch =
SBUF partition dim). Host does index preparation only (extended-label
gather of y_pred, skip masks folded into a second prob tensor PM=P*mask,
reachability mask, seam/readout fold, s-axis reversal of the forward
lattice); each core runs the CTC lattice sum in the linear-probability
domain in bf16 on the Vector engine alone (Pool/GpSimd shares SBUF ports
with DVE and contended runs measured slower than single-engine). The
forward DP (t=0..127, s-reversed) and the backward suffix DP in gamma
form (G_t = p_t*B_t, t=255..128) are FUSED side by side in one
[128, 208] tile so each DP step is 4 double-pumped bf16 instructions:
  a = X + X>>1 ; b = X>>2 * PM_t ; c = a * P_t ; X' = c + b
(the >>k reads are +k column offsets; both chains shift the same way
because the forward lattice is reversed). Guard columns between/after
the halves are re-zeroed for free by zero pad columns in P/PM. Every 8
steps each half is renormalized to 2^110 by a power-of-2 scale from its
row max. The halves meet in a one-time log-domain seam (per-lane exact
bit logs + one reversed copy + logsumexp — lane products span ~2^-175
and would underflow linear f32):
ll = m + ln Σ_s exp(lnA+lnB−m) + ln2·(Σ exponent shifts − 220 − 254).
No collectives; host concatenates the per-core [128,1] outputs.
"""

import numpy as np
import ml_dtypes

import concourse.bacc as bacc
import concourse.mybir as mybir
import concourse.tile as tile
from concourse.bass_utils import run_bass_kernel_spmd

B, T, C, U = 1024, 256, 100, 48
S = 2 * U + 1          # 97 extended-label positions
HW = 104               # half-width: S padded to 104 (208B bf16 rows)
W2 = 2 * HW            # fused row: [rev-fwd | bwd-gamma]
BLANK = C - 1
EPS = 1e-7
NCORES = 8
BS = B // NCORES       # 128 samples per core = SBUF partition dim
CH = 16                # time steps per DMA chunk
TH = 128               # fused steps + init slot
RENORM = 8
RT_LOG2 = 110          # renorm target 2^110 (also the host init scale)
EVENTS = [i for i in range(2, TH - 1) if i % RENORM == 0]  # 8..120, 15
NEV = len(EVENTS)
NLG = 2 * NEV
F32 = mybir.dt.float32
BF16 = mybir.dt.bfloat16
I32 = mybir.dt.int32
ALU = mybir.AluOpType
AXX = mybir.AxisListType.X
ACTF = mybir.ActivationFunctionType
LN2 = float(np.log(2.0))
TINY = 1e-38
DEAD = -1000.0


def _emit(nc, tc, p_d, pm_d, out_d):
    v = nc.vector
    with tc.tile_pool(name="pchunks", bufs=1) as ppool, tc.tile_pool(
        name="work", bufs=1
    ) as wp:
        ps = []
        pms = []
        for ci in range(TH // CH):
            pt_ = ppool.tile([BS, CH * W2], BF16, name=f"p{ci}")
            nc.sync.dma_start(out=pt_[:], in_=p_d[:, ci * CH : (ci + 1) * CH, :])
            ps.append(pt_)
            pmt = ppool.tile([BS, CH * W2], BF16, name=f"pm{ci}")
            nc.sync.dma_start(out=pmt[:], in_=pm_d[:, ci * CH : (ci + 1) * CH, :])
            pms.append(pmt)

        xA = wp.tile([BS, W2], BF16, name="xA")
        xB = wp.tile([BS, W2], BF16, name="xB")
        a_t = wp.tile([BS, W2 - 2], BF16, name="a_t")
        b_t = wp.tile([BS, W2 - 2], BF16, name="b_t")
        c_t = wp.tile([BS, W2 - 2], BF16, name="c_t")
        mx2 = wp.tile([BS, 2], F32, name="mx2")
        k2 = wp.tile([BS, 2], I32, name="k2")
        sc2 = wp.tile([BS, 2], I32, name="sc2")
        lgi = wp.tile([BS, NLG], I32, name="lgi")
        lgall = wp.tile([BS, NLG], F32, name="lgall")
        d_in = wp.tile([BS, 1], F32, name="d_in")
        d_out = wp.tile([BS, 1], F32, name="d_out")

        v.memset(xA[:], 0.0)
        v.memset(xB[:], 0.0)
        v.memset(lgi[:], 0)
        # preload ACT Ln/Exp tables during the DMA wait (they're used only
        # in the seam; a cold table load there would serialize ~2.6us).
        v.memset(d_in[:], 1.0)
        nc.scalar.activation(out=d_out[:], in_=d_in[:], func=ACTF.Ln)
        nc.scalar.activation(out=d_out[:], in_=d_in[:], func=ACTF.Exp, bias=0.0, scale=1.0)
        # init row (host-built): [rev alpha_0 | sel*p_255], both * 2^110.
        v.tensor_copy(out=xA[:, 0:W2], in_=ps[0][:, 0:W2])

        lg_col = {i: 2 * e for e, i in enumerate(EVENTS)}

        for i in range(1, TH):
            ci, off = i // CH, (i % CH) * W2
            pt = ps[ci][:, off : off + W2 - 2]
            pmt = pms[ci][:, off : off + W2 - 2]
            cur, nxt = (xA, xB) if i % 2 == 1 else (xB, xA)
            ev = i in EVENTS
            ap = i % RENORM == 1 and i > 1

            if ap:
                # apply last event's per-half scales to the carried state
                v.tensor_scalar(out=cur[:, 0:HW], in0=cur[:, 0:HW], scalar1=sc2[:].bitcast(F32)[:, 0:1], scalar2=None, op0=ALU.mult)
                v.tensor_scalar(out=cur[:, HW:W2], in0=cur[:, HW:W2], scalar1=sc2[:].bitcast(F32)[:, 1:2], scalar2=None, op0=ALU.mult)
            v.tensor_tensor(out=a_t[:], in0=cur[:, 0 : W2 - 2], in1=cur[:, 1 : W2 - 1], op=ALU.add)
            v.tensor_tensor(out=b_t[:], in0=cur[:, 2:W2], in1=pmt, op=ALU.mult)
            v.tensor_tensor(out=c_t[:], in0=a_t[:], in1=pt, op=ALU.mult)
            v.tensor_tensor(out=nxt[:, 0 : W2 - 2], in0=c_t[:], in1=b_t[:], op=ALU.add)

            if ev:
                c = lg_col[i]
                v.tensor_reduce(out=mx2[:, 0:1], in_=nxt[:, 0:S], axis=AXX, op=ALU.max)
                v.tensor_reduce(out=mx2[:, 1:2], in_=nxt[:, HW : HW + S], axis=AXX, op=ALU.max)
                # k = clamp(e(max)-110, -126) into lgi; sc = bits of 2^-k
                v.tensor_scalar(out=k2[:], in0=mx2[:].bitcast(I32), scalar1=23, scalar2=None, op0=ALU.logical_shift_right)
                v.tensor_scalar(out=lgi[:, c : c + 2], in0=k2[:], scalar1=127 + RT_LOG2, scalar2=-126, op0=ALU.subtract, op1=ALU.max)
                v.tensor_scalar(out=sc2[:], in0=lgi[:, c : c + 2], scalar1=-1, scalar2=127, op0=ALU.mult, op1=ALU.add)
                v.tensor_scalar(out=sc2[:], in0=sc2[:], scalar1=23, scalar2=None, op0=ALU.logical_shift_left)

        # TH-1=127 odd => final fused state lives in xB.
        xfin = xB

        # ---- seam partial: B_127 = G + G^(+1) + skip2*G^(+2), computed
        # with the same fused ops; PM slot 0 carries [0 | skip2].
        pm0 = pms[0][:, 0 : W2 - 2]
        v.tensor_tensor(out=a_t[:], in0=xfin[:, 0 : W2 - 2], in1=xfin[:, 1 : W2 - 1], op=ALU.add)
        v.tensor_tensor(out=b_t[:], in0=xfin[:, 2:W2], in1=pm0, op=ALU.mult)
        v.tensor_tensor(out=c_t[:], in0=a_t[:], in1=b_t[:], op=ALU.add)

        # ---- log-domain seam ----
        ff32 = wp.tile([BS, S], F32, name="ff32")
        bf32 = wp.tile([BS, S], F32, name="bf32")
        as_t = wp.tile([BS, S], F32, name="as_t")
        ei_t = wp.tile([BS, S], I32, name="ei_t")
        mi_t = wp.tile([BS, S], I32, name="mi_t")
        ef_t = wp.tile([BS, S], F32, name="ef_t")
        lm_t = wp.tile([BS, S], F32, name="lm_t")
        la_t = wp.tile([BS, S], F32, name="la_t")
        da_t = wp.tile([BS, S], F32, name="da_t")
        la2 = wp.tile([BS, S], F32, name="la2")
        la2r = wp.tile([BS, S], F32, name="la2r")
        lb2 = wp.tile([BS, S], F32, name="lb2")
        lam = wp.tile([BS, S], F32, name="lam")
        m_t = wp.tile([BS, 1], F32, name="m_t")
        nm_t = wp.tile([BS, 1], F32, name="nm_t")
        e_t = wp.tile([BS, S], F32, name="e_t")
        z_t = wp.tile([BS, 1], F32, name="z_t")
        lnz_t = wp.tile([BS, 1], F32, name="lnz_t")
        racc = wp.tile([BS, 1], F32, name="racc")
        acc1 = wp.tile([BS, 1], F32, name="acc1")
        acc2 = wp.tile([BS, 1], F32, name="acc2")
        outt = wp.tile([BS, 1], F32, name="outt")

        def side_log(x32, out_lam):
            # exact log of f32 x: split exponent/mantissa so the Ln LUT only
            # sees [1,2); zero lanes forced to DEAD. The biased-exponent
            # -127*ln2 per side is folded into the final constant.
            v.tensor_scalar(out=as_t[:], in0=x32[:], scalar1=1.0, scalar2=TINY, op0=ALU.mult, op1=ALU.add)
            ai = as_t[:].bitcast(I32)
            v.tensor_scalar(out=ei_t[:], in0=ai, scalar1=23, scalar2=None, op0=ALU.logical_shift_right)
            v.tensor_scalar(out=mi_t[:], in0=ai, scalar1=0x007FFFFF, scalar2=0x3F800000, op0=ALU.bitwise_and, op1=ALU.bitwise_or)
            v.tensor_copy(out=ef_t[:], in_=ei_t[:])
            nc.scalar.activation(out=lm_t[:], in_=mi_t[:].bitcast(F32), func=ACTF.Ln)
            v.scalar_tensor_tensor(out=la_t[:], in0=ef_t[:], scalar=LN2, in1=lm_t[:], op0=ALU.mult, op1=ALU.add)
            v.tensor_scalar(out=da_t[:], in0=x32[:], scalar1=0.0, scalar2=None, op0=ALU.is_equal)
            v.scalar_tensor_tensor(out=out_lam[:], in0=da_t[:], scalar=DEAD, in1=la_t[:], op0=ALU.mult, op1=ALU.add)

        v.tensor_copy(out=ff32[:], in_=xfin[:, 0:S])
        side_log(ff32, la2)
        v.tensor_copy(out=bf32[:], in_=c_t[:, HW : HW + S])
        side_log(bf32, lb2)
        # un-reverse the fwd side so lanes line up with the bwd side
        v.tensor_copy(out=la2r[:, 0:S], in_=la2[:, S - 1 :: -1])
        v.tensor_tensor(out=lam[:], in0=la2r[:], in1=lb2[:], op=ALU.add)
        v.tensor_reduce(out=m_t[:], in_=lam[:], axis=AXX, op=ALU.max)
        v.tensor_scalar(out=nm_t[:], in0=m_t[:], scalar1=-1.0, scalar2=None, op0=ALU.mult)
        nc.scalar.activation(out=e_t[:], in_=lam[:], func=ACTF.Exp, bias=nm_t[:, 0:1], scale=1.0)
        v.tensor_reduce(out=z_t[:], in_=e_t[:], axis=AXX, op=ALU.add)
        nc.scalar.activation(out=lnz_t[:], in_=z_t[:], func=ACTF.Ln)
        v.tensor_copy(out=lgall[:], in_=lgi[:])
        v.tensor_reduce(out=racc[:], in_=lgall[:], axis=AXX, op=ALU.add)
        # constants: -2*RT for the host 2^110 injections, -2*127 for the two
        # side_log biased exponents.
        v.tensor_scalar(out=racc[:], in0=racc[:], scalar1=float(-2 * RT_LOG2 - 254), scalar2=None, op0=ALU.add)
        v.scalar_tensor_tensor(out=acc1[:], in0=racc[:], scalar=LN2, in1=m_t[:], op0=ALU.mult, op1=ALU.add)
        v.tensor_tensor(out=acc2[:], in0=acc1[:], in1=lnz_t[:], op=ALU.add)
        nc.scalar.mul(out=outt[:], in_=acc2[:], mul=-1.0)
        nc.sync.dma_start(out=out_d, in_=outt[:])


def _build_program():
    nc = bacc.Bacc("TRN2", target_bir_lowering=False, debug=False)
    p_d = nc.dram_tensor("p", [BS, TH, W2], BF16, kind="ExternalInput").ap()
    pm_d = nc.dram_tensor("pm", [BS, TH, W2], BF16, kind="ExternalInput").ap()
    out_d = nc.dram_tensor("out", [BS, 1], F32, kind="ExternalOutput").ap()
    with tile.TileContext(nc) as tc:
        _emit(nc, tc, p_d, pm_d, out_d)
    nc.compile()
    return nc


_NC = None


def _get_nc():
    global _NC
    if _NC is None:
        _NC = _build_program()
    return _NC


def _prep_in_maps(y_pred, y_true, label_length):
    ext = np.full((B, S), BLANK, np.int32)
    ext[:, 1::2] = y_true.astype(np.int32)
    prev2 = np.concatenate([np.full((B, 2), BLANK, np.int32), ext[:, :-2]], axis=1)
    skip = ((ext != BLANK) & (ext != prev2)).astype(np.float32)
    skip2 = np.concatenate([skip[:, 2:], np.zeros((B, 2), np.float32)], axis=1)
    P = np.take_along_axis(
        np.ascontiguousarray(y_pred, dtype=np.float32), ext[:, None, :], axis=2
    )
    P += np.float32(EPS)
    L = label_length.reshape(B).astype(np.int64)
    i2 = np.clip(2 * L, 0, S - 1)
    i1 = np.maximum(i2 - 1, 0)
    # reachability mask: position s at time t is dead if it cannot reach i1
    # by t=T-1 (max +2 per step). Folded into P at zero device cost.
    s_idx = np.arange(S)[None, None, :]
    t_idx = np.arange(T)[None, :, None]
    alive = (s_idx + 2 * (T - 1 - t_idx)) >= i1[:, None, None]
    P *= alive.astype(np.float32)
    sel = np.zeros((B, S), np.float32)
    sel[np.arange(B), i1] = 1.0
    sel[np.arange(B), i2] = 1.0
    SCALE = np.float32(2.0**RT_LOG2)
    # fused rows: left = s-reversed fwd probs, right = bwd probs (t falling)
    pfull = np.zeros((B, TH, W2), np.float32)
    pmfull = np.zeros((B, TH, W2), np.float32)
    pfull[:, 1:, 0:S] = P[:, 1:TH, ::-1]
    pfull[:, 1:, HW : HW + S] = P[:, T - 2 : T - 1 - TH : -1, :]
    skr = skip[:, ::-1]
    pmfull[:, 1:, 0:S] = pfull[:, 1:, 0:S] * skr[:, None, :]
    pmfull[:, 1:, HW : HW + S] = pfull[:, 1:, HW : HW + S] * skip2[:, None, :]
    # init row: [rev alpha_0 | sel*p_255] * 2^110
    pfull[:, 0, S - 2] = P[:, 0, 1] * SCALE
    pfull[:, 0, S - 1] = P[:, 0, 0] * SCALE
    pfull[:, 0, HW : HW + S] = sel * P[:, T - 1, :] * SCALE
    # PM slot 0 = seam mask [0 | skip2] for the final partial step
    pmfull[:, 0, HW : HW + S] = skip2
    pfull = pfull.astype(ml_dtypes.bfloat16)
    pmfull = pmfull.astype(ml_dtypes.bfloat16)
    in_maps = []
    for c in range(NCORES):
        sl = slice(c * BS, (c + 1) * BS)
        in_maps.append(
            {
                "p": np.ascontiguousarray(pfull[sl]),
                "pm": np.ascontiguousarray(pmfull[sl]),
            }
        )
    return in_maps


def _run_device(in_maps, **kwargs):
    nc = _get_nc()
    return run_bass_kernel_spmd(nc, in_maps, core_ids=list(range(NCORES)), **kwargs)


def _ctc_numpy(y_pred, y_true, input_length, label_length):
    """Generality safety net (log domain, mirrors the reference exactly)."""
    b, t_max, c = y_pred.shape
    u = y_true.shape[1]
    s = 2 * u + 1
    blank = c - 1
    neg = np.float32(-1e30)
    logp = np.log(y_pred.astype(np.float32) + np.float32(EPS))
    ext = np.full((b, s), blank, np.int32)
    ext[:, 1::2] = y_true.astype(np.int32)
    prev2 = np.concatenate([np.full((b, 2), blank, np.int32), ext[:, :-2]], axis=1)
    can_skip = (ext != blank) & (ext != prev2)
    lp_ext = np.take_along_axis(logp, ext[:, None, :], axis=2)
    alpha = np.full((b, s), neg, np.float32)
    alpha[:, 0] = lp_ext[:, 0, 0]
    alpha[:, 1] = lp_ext[:, 0, 1]
    inp_len = input_length.reshape(b)

    def lse(stack):
        m = np.max(stack, axis=0)
        return m + np.log(np.sum(np.exp(stack - m), axis=0))

    for t in range(1, t_max):
        a1 = np.concatenate([np.full((b, 1), neg, np.float32), alpha[:, :-1]], axis=1)
        a2 = np.concatenate([np.full((b, 2), neg, np.float32), alpha[:, :-2]], axis=1)
        a2 = np.where(can_skip, a2, neg)
        new = lse(np.stack([alpha, a1, a2], 0)).astype(np.float32) + lp_ext[:, t, :]
        alpha = np.where((t < inp_len)[:, None], new, alpha)
    L = label_length.reshape(b).astype(np.int64)
    i2 = np.clip(2 * L, 0, s - 1)
    i1 = np.maximum(i2 - 1, 0)
    a_last = np.stack([alpha[np.arange(b), i1], alpha[np.arange(b), i2]], axis=1)
    ll = np.where(L > 0, lse(a_last.T).astype(np.float32), alpha[:, 0])
    return (-ll[:, None]).astype(np.float32)


def kernel(y_pred, y_true, input_length, label_length):
    y_pred = np.asarray(y_pred)
    y_true = np.asarray(y_true)
    input_length = np.asarray(input_length)
    label_length = np.asarray(label_length)
    if y_pred.shape != (B, T, C) or y_true.shape != (B, U) or not np.all(
        input_length.reshape(-1) == T
    ) or np.any(label_length.reshape(-1) <= 0):
        return _ctc_numpy(y_pred, y_true, input_length, label_length)
    in_maps = _prep_in_maps(y_pred, y_true, label_length)
    res = _run_device(in_maps)
    out = np.concatenate([r["out"] for r in res.results], axis=0)
    return np.ascontiguousarray(out, dtype=np.float32)


# revision 13
# speedup vs baseline: 1.9895x; 1.1273x over previous
"""CTC loss (Keras ctc_batch_cost semantics) on 8 Trainium2 NeuronCores.

Strategy: pure data parallelism — batch B=1024 sharded 128/core (batch =
SBUF partition dim). Host does index preparation only (extended-label
gather of y_pred, skip masks folded into a second prob tensor PM=P*mask,
reachability mask, seam/readout fold, s-axis reversal of the forward
lattice); each core runs the CTC lattice sum in the linear-probability
domain in bf16 on the Vector engine alone (Pool/GpSimd shares SBUF ports
with DVE and contended runs measured slower than single-engine). The
forward DP (t=0..127, s-reversed) and the backward suffix DP in gamma
form (G_t = p_t*B_t, t=255..128) are FUSED side by side in one
[128, 208] tile so each DP step is 4 double-pumped bf16 instructions:
  a = X + X>>1 ; b = X>>2 * PM_t ; c = a * P_t ; X' = c + b
(the >>k reads are +k column offsets; both chains shift the same way
because the forward lattice is reversed). Guard columns between/after
the halves are re-zeroed for free by zero pad columns in P/PM. Every 8
steps each half is renormalized to 2^110 by a power-of-2 scale from its
row max. The halves meet in a one-time log-domain seam (per-lane exact
bit logs + one reversed copy + logsumexp — lane products span ~2^-175
and would underflow linear f32):
ll = m + ln Σ_s exp(lnA+lnB−m) + ln2·(Σ exponent shifts − 220 − 254).
No collectives; host concatenates the per-core [128,1] outputs.
"""

import numpy as np
import ml_dtypes

import concourse.bacc as bacc
import concourse.mybir as mybir
import concourse.tile as tile
from concourse.bass_utils import run_bass_kernel_spmd

B, T, C, U = 1024, 256, 100, 48
S = 2 * U + 1          # 97 extended-label positions
HW = 104               # half-width: S padded to 104 (208B bf16 rows)
W2 = 2 * HW            # fused row: [rev-fwd | bwd-gamma]
BLANK = C - 1
EPS = 1e-7
NCORES = 8
BS = B // NCORES       # 128 samples per core = SBUF partition dim
CH = 16                # time steps per DMA chunk
TH = 128               # fused steps + init slot
RENORM = 24            # host prescale bounds drift to ~75 bits/24 steps
RT_LOG2 = 120          # renorm target 2^120 (also the host init scale)
EVENTS = [i for i in range(2, TH - 1) if i % RENORM == 0]  # 8..120, 15
NEV = len(EVENTS)
NLG = 2 * NEV
F32 = mybir.dt.float32
BF16 = mybir.dt.bfloat16
I32 = mybir.dt.int32
ALU = mybir.AluOpType
AXX = mybir.AxisListType.X
ACTF = mybir.ActivationFunctionType
LN2 = float(np.log(2.0))
TINY = 1e-38
DEAD = -1000.0


def _emit(nc, tc, p_d, pm_d, out_d):
    v = nc.vector
    with tc.tile_pool(name="pchunks", bufs=1) as ppool, tc.tile_pool(
        name="work", bufs=1
    ) as wp:
        ps = []
        pms = []
        for ci in range(TH // CH):
            pt_ = ppool.tile([BS, CH * W2], BF16, name=f"p{ci}")
            pmt = ppool.tile([BS, CH * W2], BF16, name=f"pm{ci}")
            if ci == 0:
                # split the first chunks across queues: the first DP step
                # waits on them, later chunks hide behind compute.
                for j in range(4):
                    sl = slice(j * 4 * W2, (j + 1) * 4 * W2)
                    nc.sync.dma_start(out=pt_[:, sl], in_=p_d[:, j * 4 : (j + 1) * 4, :])
                    nc.sync.dma_start(out=pmt[:, sl], in_=pm_d[:, j * 4 : (j + 1) * 4, :])
            else:
                nc.sync.dma_start(out=pt_[:], in_=p_d[:, ci * CH : (ci + 1) * CH, :])
                nc.sync.dma_start(out=pmt[:], in_=pm_d[:, ci * CH : (ci + 1) * CH, :])
            ps.append(pt_)
            pms.append(pmt)

        xA = wp.tile([BS, W2], BF16, name="xA")
        xB = wp.tile([BS, W2], BF16, name="xB")
        a_t = wp.tile([BS, W2 - 2], BF16, name="a_t")
        b_t = wp.tile([BS, W2 - 2], BF16, name="b_t")
        c_t = wp.tile([BS, W2 - 2], BF16, name="c_t")
        mx2 = wp.tile([BS, 2], F32, name="mx2")
        k2 = wp.tile([BS, 2], I32, name="k2")
        sc2 = wp.tile([BS, 2], I32, name="sc2")
        lgi = wp.tile([BS, NLG], I32, name="lgi")
        lgall = wp.tile([BS, NLG], F32, name="lgall")
        d_in = wp.tile([BS, 1], F32, name="d_in")
        d_out = wp.tile([BS, 1], F32, name="d_out")

        v.memset(xA[:], 0.0)
        v.memset(xB[:], 0.0)
        v.memset(lgi[:], 0)
        # preload ACT Ln/Exp tables during the DMA wait (they're used only
        # in the seam; a cold table load there would serialize ~2.6us).
        v.memset(d_in[:], 1.0)
        nc.scalar.activation(out=d_out[:], in_=d_in[:], func=ACTF.Ln)
        # init row (host-built): [rev alpha_0 | sel*p_255], both * 2^110.
        v.tensor_copy(out=xA[:, 0:W2], in_=ps[0][:, 0:W2])

        lg_col = {i: 2 * e for e, i in enumerate(EVENTS)}

        for i in range(1, TH):
            ci, off = i // CH, (i % CH) * W2
            pt = ps[ci][:, off : off + W2 - 2]
            pmt = pms[ci][:, off : off + W2 - 2]
            cur, nxt = (xA, xB) if i % 2 == 1 else (xB, xA)
            ev = i in EVENTS
            ap = i % RENORM == 1 and i > 1

            if ap:
                # apply last event's per-half scales to the carried state
                v.tensor_scalar(out=cur[:, 0:HW], in0=cur[:, 0:HW], scalar1=sc2[:].bitcast(F32)[:, 0:1], scalar2=None, op0=ALU.mult)
                v.tensor_scalar(out=cur[:, HW:W2], in0=cur[:, HW:W2], scalar1=sc2[:].bitcast(F32)[:, 1:2], scalar2=None, op0=ALU.mult)
            v.tensor_tensor(out=a_t[:], in0=cur[:, 0 : W2 - 2], in1=cur[:, 1 : W2 - 1], op=ALU.add)
            v.tensor_tensor(out=b_t[:], in0=cur[:, 2:W2], in1=pmt, op=ALU.mult)
            v.tensor_tensor(out=c_t[:], in0=a_t[:], in1=pt, op=ALU.mult)
            v.tensor_tensor(out=nxt[:, 0 : W2 - 2], in0=c_t[:], in1=b_t[:], op=ALU.add)

            if ev:
                c = lg_col[i]
                v.tensor_reduce(out=mx2[:, 0:1], in_=nxt[:, 0:S], axis=AXX, op=ALU.max)
                v.tensor_reduce(out=mx2[:, 1:2], in_=nxt[:, HW : HW + S], axis=AXX, op=ALU.max)
                # k = clamp(e(max)-110, -126) into lgi; sc = bits of 2^-k
                v.tensor_scalar(out=k2[:], in0=mx2[:].bitcast(I32), scalar1=23, scalar2=None, op0=ALU.logical_shift_right)
                v.tensor_scalar(out=lgi[:, c : c + 2], in0=k2[:], scalar1=127 + RT_LOG2, scalar2=-126, op0=ALU.subtract, op1=ALU.max)
                v.tensor_scalar(out=sc2[:], in0=lgi[:, c : c + 2], scalar1=-1, scalar2=127, op0=ALU.mult, op1=ALU.add)
                v.tensor_scalar(out=sc2[:], in0=sc2[:], scalar1=23, scalar2=None, op0=ALU.logical_shift_left)

        # TH-1=127 odd => final fused state lives in xB.
        xfin = xB

        # ---- seam partial: B_127 = G + G^(+1) + skip2*G^(+2), computed
        # with the same fused ops; PM slot 0 carries [0 | skip2].
        pm0 = pms[0][:, 0 : W2 - 2]
        v.tensor_tensor(out=a_t[:], in0=xfin[:, 0 : W2 - 2], in1=xfin[:, 1 : W2 - 1], op=ALU.add)
        v.tensor_tensor(out=b_t[:], in0=xfin[:, 2:W2], in1=pm0, op=ALU.mult)
        v.tensor_tensor(out=c_t[:], in0=a_t[:], in1=b_t[:], op=ALU.add)

        # ---- log-domain seam ----
        ff32 = wp.tile([BS, S], F32, name="ff32")
        bf32 = wp.tile([BS, S], F32, name="bf32")
        as_t = wp.tile([BS, S], F32, name="as_t")
        ei_t = wp.tile([BS, S], I32, name="ei_t")
        mi_t = wp.tile([BS, S], I32, name="mi_t")
        ef_t = wp.tile([BS, S], F32, name="ef_t")
        lm_t = wp.tile([BS, S], F32, name="lm_t")
        la_t = wp.tile([BS, S], F32, name="la_t")
        da_t = wp.tile([BS, S], F32, name="da_t")
        la2 = wp.tile([BS, S], F32, name="la2")
        la2r = wp.tile([BS, S], F32, name="la2r")
        lb2 = wp.tile([BS, S], F32, name="lb2")
        lam = wp.tile([BS, S], F32, name="lam")
        m_t = wp.tile([BS, 1], F32, name="m_t")
        nm_t = wp.tile([BS, 1], F32, name="nm_t")
        e_t = wp.tile([BS, S], F32, name="e_t")
        z_t = wp.tile([BS, 1], F32, name="z_t")
        lnz_t = wp.tile([BS, 1], F32, name="lnz_t")
        racc = wp.tile([BS, 1], F32, name="racc")
        acc1 = wp.tile([BS, 1], F32, name="acc1")
        acc2 = wp.tile([BS, 1], F32, name="acc2")
        outt = wp.tile([BS, 1], F32, name="outt")

        def side_log(x32, out_lam):
            # exact log of f32 x: split exponent/mantissa so the Ln LUT only
            # sees [1,2); zero lanes forced to DEAD. The biased-exponent
            # -127*ln2 per side is folded into the final constant.
            v.tensor_scalar(out=as_t[:], in0=x32[:], scalar1=1.0, scalar2=TINY, op0=ALU.mult, op1=ALU.add)
            ai = as_t[:].bitcast(I32)
            v.tensor_scalar(out=ei_t[:], in0=ai, scalar1=23, scalar2=None, op0=ALU.logical_shift_right)
            v.tensor_scalar(out=mi_t[:], in0=ai, scalar1=0x007FFFFF, scalar2=0x3F800000, op0=ALU.bitwise_and, op1=ALU.bitwise_or)
            v.tensor_copy(out=ef_t[:], in_=ei_t[:])
            nc.scalar.activation(out=lm_t[:], in_=mi_t[:].bitcast(F32), func=ACTF.Ln)
            v.scalar_tensor_tensor(out=la_t[:], in0=ef_t[:], scalar=LN2, in1=lm_t[:], op0=ALU.mult, op1=ALU.add)
            v.tensor_scalar(out=da_t[:], in0=x32[:], scalar1=0.0, scalar2=None, op0=ALU.is_equal)
            v.scalar_tensor_tensor(out=out_lam[:], in0=da_t[:], scalar=DEAD, in1=la_t[:], op0=ALU.mult, op1=ALU.add)

        v.tensor_copy(out=ff32[:], in_=xfin[:, 0:S])
        side_log(ff32, la2)
        v.tensor_copy(out=bf32[:], in_=c_t[:, HW : HW + S])
        side_log(bf32, lb2)
        # un-reverse the fwd side so lanes line up with the bwd side
        v.tensor_copy(out=la2r[:, 0:S], in_=la2[:, S - 1 :: -1])
        v.tensor_tensor(out=lam[:], in0=la2r[:], in1=lb2[:], op=ALU.add)
        v.tensor_reduce(out=m_t[:], in_=lam[:], axis=AXX, op=ALU.max)
        v.tensor_scalar(out=nm_t[:], in0=m_t[:], scalar1=-1.0, scalar2=None, op0=ALU.mult)
        nc.scalar.activation(out=e_t[:], in_=lam[:], func=ACTF.Exp, bias=nm_t[:, 0:1], scale=1.0)
        v.tensor_reduce(out=z_t[:], in_=e_t[:], axis=AXX, op=ALU.add)
        nc.scalar.activation(out=lnz_t[:], in_=z_t[:], func=ACTF.Ln)
        v.tensor_copy(out=lgall[:], in_=lgi[:])
        v.tensor_reduce(out=racc[:], in_=lgall[:], axis=AXX, op=ALU.add)
        # constants: -2*RT for the host 2^110 injections, -2*127 for the two
        # side_log biased exponents.
        v.tensor_scalar(out=racc[:], in0=racc[:], scalar1=float(-2 * RT_LOG2 - 254), scalar2=None, op0=ALU.add)
        v.scalar_tensor_tensor(out=acc1[:], in0=racc[:], scalar=LN2, in1=m_t[:], op0=ALU.mult, op1=ALU.add)
        v.tensor_tensor(out=acc2[:], in0=acc1[:], in1=lnz_t[:], op=ALU.add)
        v.tensor_scalar(out=outt[:], in0=acc2[:], scalar1=-1.0, scalar2=None, op0=ALU.mult)
        nc.sync.dma_start(out=out_d, in_=outt[:])


def _build_program():
    nc = bacc.Bacc("TRN2", target_bir_lowering=False, debug=False)
    p_d = nc.dram_tensor("p", [BS, TH, W2], BF16, kind="ExternalInput").ap()
    pm_d = nc.dram_tensor("pm", [BS, TH, W2], BF16, kind="ExternalInput").ap()
    out_d = nc.dram_tensor("out", [BS, 1], F32, kind="ExternalOutput").ap()
    with tile.TileContext(nc) as tc:
        _emit(nc, tc, p_d, pm_d, out_d)
    nc.compile()
    return nc


_NC = None


def _get_nc():
    global _NC
    if _NC is None:
        _NC = _build_program()
    return _NC


def _prep_in_maps(y_pred, y_true, label_length):
    ext = np.full((B, S), BLANK, np.int32)
    ext[:, 1::2] = y_true.astype(np.int32)
    prev2 = np.concatenate([np.full((B, 2), BLANK, np.int32), ext[:, :-2]], axis=1)
    skip = ((ext != BLANK) & (ext != prev2)).astype(np.float32)
    skip2 = np.concatenate([skip[:, 2:], np.zeros((B, 2), np.float32)], axis=1)
    P = np.take_along_axis(
        np.ascontiguousarray(y_pred, dtype=np.float32), ext[:, None, :], axis=2
    )
    P += np.float32(EPS)
    L = label_length.reshape(B).astype(np.int64)
    i2 = np.clip(2 * L, 0, S - 1)
    i1 = np.maximum(i2 - 1, 0)
    # reachability mask: position s at time t is dead if it cannot reach i1
    # by t=T-1 (max +2 per step). Folded into P at zero device cost.
    s_idx = np.arange(S)[None, None, :]
    t_idx = np.arange(T)[None, :, None]
    alive = (s_idx + 2 * (T - 1 - t_idx)) >= i1[:, None, None]
    P *= alive.astype(np.float32)
    # host prescale: scale each (b,t) row to max ~1 (exact powers of two);
    # the exact correction Sum k_t * ln2 is added back on the host.
    k_t = np.round(np.log2(P.max(2))).astype(np.int32)
    P = P * np.exp2(-k_t.astype(np.float64))[:, :, None].astype(np.float32)
    kcorr = k_t.sum(1).astype(np.float64)
    sel = np.zeros((B, S), np.float32)
    sel[np.arange(B), i1] = 1.0
    sel[np.arange(B), i2] = 1.0
    SCALE = np.float32(2.0**RT_LOG2)
    # fused rows: left = s-reversed fwd probs, right = bwd probs (t falling)
    pfull = np.zeros((B, TH, W2), np.float32)
    pmfull = np.zeros((B, TH, W2), np.float32)
    pfull[:, 1:, 0:S] = P[:, 1:TH, ::-1]
    pfull[:, 1:, HW : HW + S] = P[:, T - 2 : T - 1 - TH : -1, :]
    skr = skip[:, ::-1]
    pmfull[:, 1:, 0:S] = pfull[:, 1:, 0:S] * skr[:, None, :]
    pmfull[:, 1:, HW : HW + S] = pfull[:, 1:, HW : HW + S] * skip2[:, None, :]
    # init row: [rev alpha_0 | sel*p_255] * 2^110
    pfull[:, 0, S - 2] = P[:, 0, 1] * SCALE
    pfull[:, 0, S - 1] = P[:, 0, 0] * SCALE
    pfull[:, 0, HW : HW + S] = sel * P[:, T - 1, :] * SCALE
    # PM slot 0 = seam mask [0 | skip2] for the final partial step
    pmfull[:, 0, HW : HW + S] = skip2
    pfull = pfull.astype(ml_dtypes.bfloat16)
    pmfull = pmfull.astype(ml_dtypes.bfloat16)
    in_maps = []
    for c in range(NCORES):
        sl = slice(c * BS, (c + 1) * BS)
        in_maps.append(
            {
                "p": np.ascontiguousarray(pfull[sl]),
                "pm": np.ascontiguousarray(pmfull[sl]),
            }
        )
    return in_maps, kcorr


def _run_device(in_maps, **kwargs):
    nc = _get_nc()
    return run_bass_kernel_spmd(nc, in_maps, core_ids=list(range(NCORES)), **kwargs)


def _ctc_numpy(y_pred, y_true, input_length, label_length):
    """Generality safety net (log domain, mirrors the reference exactly)."""
    b, t_max, c = y_pred.shape
    u = y_true.shape[1]
    s = 2 * u + 1
    blank = c - 1
    neg = np.float32(-1e30)
    logp = np.log(y_pred.astype(np.float32) + np.float32(EPS))
    ext = np.full((b, s), blank, np.int32)
    ext[:, 1::2] = y_true.astype(np.int32)
    prev2 = np.concatenate([np.full((b, 2), blank, np.int32), ext[:, :-2]], axis=1)
    can_skip = (ext != blank) & (ext != prev2)
    lp_ext = np.take_along_axis(logp, ext[:, None, :], axis=2)
    alpha = np.full((b, s), neg, np.float32)
    alpha[:, 0] = lp_ext[:, 0, 0]
    alpha[:, 1] = lp_ext[:, 0, 1]
    inp_len = input_length.reshape(b)

    def lse(stack):
        m = np.max(stack, axis=0)
        return m + np.log(np.sum(np.exp(stack - m), axis=0))

    for t in range(1, t_max):
        a1 = np.concatenate([np.full((b, 1), neg, np.float32), alpha[:, :-1]], axis=1)
        a2 = np.concatenate([np.full((b, 2), neg, np.float32), alpha[:, :-2]], axis=1)
        a2 = np.where(can_skip, a2, neg)
        new = lse(np.stack([alpha, a1, a2], 0)).astype(np.float32) + lp_ext[:, t, :]
        alpha = np.where((t < inp_len)[:, None], new, alpha)
    L = label_length.reshape(b).astype(np.int64)
    i2 = np.clip(2 * L, 0, s - 1)
    i1 = np.maximum(i2 - 1, 0)
    a_last = np.stack([alpha[np.arange(b), i1], alpha[np.arange(b), i2]], axis=1)
    ll = np.where(L > 0, lse(a_last.T).astype(np.float32), alpha[:, 0])
    return (-ll[:, None]).astype(np.float32)


def kernel(y_pred, y_true, input_length, label_length):
    y_pred = np.asarray(y_pred)
    y_true = np.asarray(y_true)
    input_length = np.asarray(input_length)
    label_length = np.asarray(label_length)
    if y_pred.shape != (B, T, C) or y_true.shape != (B, U) or not np.all(
        input_length.reshape(-1) == T
    ) or np.any(label_length.reshape(-1) <= 0):
        return _ctc_numpy(y_pred, y_true, input_length, label_length)
    in_maps, kcorr = _prep_in_maps(y_pred, y_true, label_length)
    res = _run_device(in_maps)
    out = np.concatenate([r["out"] for r in res.results], axis=0)
    out = out - (LN2 * kcorr)[:, None].astype(np.float32)
    return np.ascontiguousarray(out, dtype=np.float32)


# revision 15
# speedup vs baseline: 1.9902x; 1.0004x over previous
"""CTC loss (Keras ctc_batch_cost semantics) on 8 Trainium2 NeuronCores.

Strategy: pure data parallelism — batch B=1024 sharded 128/core (batch =
SBUF partition dim). Host does index preparation only (extended-label
gather of y_pred, skip masks folded into a second prob tensor PM=P*mask,
reachability mask, seam/readout fold, s-axis reversal of the forward
lattice); each core runs the CTC lattice sum in the linear-probability
domain in bf16 on the Vector engine alone (Pool/GpSimd shares SBUF ports
with DVE and contended runs measured slower than single-engine). The
forward DP (t=0..127, s-reversed) and the backward suffix DP in gamma
form (G_t = p_t*B_t, t=255..128) are FUSED side by side in one
[128, 208] tile so each DP step is 4 double-pumped bf16 instructions:
  a = X + X>>1 ; b = X>>2 * PM_t ; c = a * P_t ; X' = c + b
(the >>k reads are +k column offsets; both chains shift the same way
because the forward lattice is reversed). Guard columns between/after
the halves are re-zeroed for free by zero pad columns in P/PM. The host
prescales every (b,t) row of P by a power of two so its max is ~1, which
bounds state drift to ~75 bits per 24 steps; every 24 steps each half is
renormalized to 2^120 by a power-of-2 scale from its row max, and the
host adds the exact prescale correction ln2*sum(k_t) back to the loss.
The halves meet in a one-time log-domain seam (per-lane exact bit logs
+ one reversed copy + logsumexp — lane products span ~2^-175 and would
underflow linear f32):
ll = m + ln Σ_s exp(lnA+lnB−m) + ln2·(Σ exponent shifts − 2·120 − 254).
No collectives; host concatenates the per-core [128,1] outputs.
"""

import numpy as np
import ml_dtypes

import concourse.bacc as bacc
import concourse.mybir as mybir
import concourse.tile as tile
from concourse.bass_utils import run_bass_kernel_spmd

B, T, C, U = 1024, 256, 100, 48
S = 2 * U + 1          # 97 extended-label positions
HW = 104               # half-width: S padded to 104 (208B bf16 rows)
W2 = 2 * HW            # fused row: [rev-fwd | bwd-gamma]
BLANK = C - 1
EPS = 1e-7
NCORES = 8
BS = B // NCORES       # 128 samples per core = SBUF partition dim
CH = 16                # time steps per DMA chunk
TH = 128               # fused steps + init slot
RENORM = 24            # host prescale bounds drift to ~75 bits/24 steps
RT_LOG2 = 120          # renorm target 2^120 (also the host init scale)
EVENTS = [i for i in range(2, TH - 1) if i % RENORM == 0]  # 24..120, 5
NEV = len(EVENTS)
NLG = 2 * NEV
F32 = mybir.dt.float32
BF16 = mybir.dt.bfloat16
I32 = mybir.dt.int32
ALU = mybir.AluOpType
AXX = mybir.AxisListType.X
ACTF = mybir.ActivationFunctionType
LN2 = float(np.log(2.0))
TINY = 1e-38
DEAD = -1000.0


def _emit(nc, tc, p_d, pm_d, out_d):
    v = nc.vector
    with tc.tile_pool(name="pchunks", bufs=1) as ppool, tc.tile_pool(
        name="work", bufs=1
    ) as wp:
        ps = []
        pms = []
        for ci in range(TH // CH):
            pt_ = ppool.tile([BS, CH * W2], BF16, name=f"p{ci}")
            pmt = ppool.tile([BS, CH * W2], BF16, name=f"pm{ci}")
            if ci == 0:
                # split the first chunks across queues: the first DP step
                # waits on them, later chunks hide behind compute.
                for j in range(4):
                    sl = slice(j * 4 * W2, (j + 1) * 4 * W2)
                    nc.sync.dma_start(out=pt_[:, sl], in_=p_d[:, j * 4 : (j + 1) * 4, :])
                    nc.sync.dma_start(out=pmt[:, sl], in_=pm_d[:, j * 4 : (j + 1) * 4, :])
            else:
                nc.sync.dma_start(out=pt_[:], in_=p_d[:, ci * CH : (ci + 1) * CH, :])
                nc.sync.dma_start(out=pmt[:], in_=pm_d[:, ci * CH : (ci + 1) * CH, :])
            ps.append(pt_)
            pms.append(pmt)

        xA = wp.tile([BS, W2], BF16, name="xA")
        xB = wp.tile([BS, W2], BF16, name="xB")
        a_t = wp.tile([BS, W2 - 2], BF16, name="a_t")
        b_t = wp.tile([BS, W2 - 2], BF16, name="b_t")
        c_t = wp.tile([BS, W2 - 2], BF16, name="c_t")
        mx2 = wp.tile([BS, 2], F32, name="mx2")
        k2 = wp.tile([BS, 2], I32, name="k2")
        sc2 = wp.tile([BS, 2], I32, name="sc2")
        lgi = wp.tile([BS, NLG], I32, name="lgi")
        lgall = wp.tile([BS, NLG], F32, name="lgall")
        d_in = wp.tile([BS, 1], F32, name="d_in")
        d_out = wp.tile([BS, 1], F32, name="d_out")

        v.memset(xA[:], 0.0)
        v.memset(xB[:], 0.0)
        v.memset(lgi[:], 0)
        # preload the ACT Ln table during the DMA wait (first seam use
        # would otherwise serialize a ~1.3us table load; Exp still loads
        # once in the seam — preloading it here would evict Ln).
        v.memset(d_in[:], 1.0)
        nc.scalar.activation(out=d_out[:], in_=d_in[:], func=ACTF.Ln)
        # init row (host-built): [rev alpha_0 | sel*p_255], both * 2^RT.
        v.tensor_copy(out=xA[:, 0:W2], in_=ps[0][:, 0:W2])

        lg_col = {i: 2 * e for e, i in enumerate(EVENTS)}

        for i in range(1, TH):
            ci, off = i // CH, (i % CH) * W2
            pt = ps[ci][:, off : off + W2 - 2]
            pmt = pms[ci][:, off : off + W2 - 2]
            cur, nxt = (xA, xB) if i % 2 == 1 else (xB, xA)
            ev = i in EVENTS
            ap = i % RENORM == 1 and i > 1

            if ap:
                # apply last event's per-half scales to the carried state
                v.tensor_scalar(out=cur[:, 0:HW], in0=cur[:, 0:HW], scalar1=sc2[:].bitcast(F32)[:, 0:1], scalar2=None, op0=ALU.mult)
                v.tensor_scalar(out=cur[:, HW:W2], in0=cur[:, HW:W2], scalar1=sc2[:].bitcast(F32)[:, 1:2], scalar2=None, op0=ALU.mult)
            v.tensor_tensor(out=a_t[:], in0=cur[:, 0 : W2 - 2], in1=cur[:, 1 : W2 - 1], op=ALU.add)
            v.tensor_tensor(out=b_t[:], in0=cur[:, 2:W2], in1=pmt, op=ALU.mult)
            v.tensor_tensor(out=c_t[:], in0=a_t[:], in1=pt, op=ALU.mult)
            v.tensor_tensor(out=nxt[:, 0 : W2 - 2], in0=c_t[:], in1=b_t[:], op=ALU.add)

            if ev:
                c = lg_col[i]
                v.tensor_reduce(out=mx2[:, 0:1], in_=nxt[:, 0:S], axis=AXX, op=ALU.max)
                v.tensor_reduce(out=mx2[:, 1:2], in_=nxt[:, HW : HW + S], axis=AXX, op=ALU.max)
                # k = clamp(e(max)-RT, -126) into lgi; sc = bits of 2^-k
                v.tensor_scalar(out=k2[:], in0=mx2[:].bitcast(I32), scalar1=23, scalar2=None, op0=ALU.logical_shift_right)
                v.tensor_scalar(out=lgi[:, c : c + 2], in0=k2[:], scalar1=127 + RT_LOG2, scalar2=-126, op0=ALU.subtract, op1=ALU.max)
                v.tensor_scalar(out=sc2[:], in0=lgi[:, c : c + 2], scalar1=-1, scalar2=127, op0=ALU.mult, op1=ALU.add)
                v.tensor_scalar(out=sc2[:], in0=sc2[:], scalar1=23, scalar2=None, op0=ALU.logical_shift_left)

        # TH-1=127 odd => final fused state lives in xB.
        xfin = xB

        # ---- seam partial: B_127 = G + G^(+1) + skip2*G^(+2), computed
        # with the same fused ops; PM slot 0 carries [0 | skip2].
        pm0 = pms[0][:, 0 : W2 - 2]
        v.tensor_tensor(out=a_t[:], in0=xfin[:, 0 : W2 - 2], in1=xfin[:, 1 : W2 - 1], op=ALU.add)
        v.tensor_tensor(out=b_t[:], in0=xfin[:, 2:W2], in1=pm0, op=ALU.mult)
        v.tensor_tensor(out=c_t[:], in0=a_t[:], in1=b_t[:], op=ALU.add)

        # ---- log-domain seam ----
        ff32 = wp.tile([BS, S], F32, name="ff32")
        bf32 = wp.tile([BS, S], F32, name="bf32")
        as_t = wp.tile([BS, S], F32, name="as_t")
        ei_t = wp.tile([BS, S], I32, name="ei_t")
        mi_t = wp.tile([BS, S], I32, name="mi_t")
        ef_t = wp.tile([BS, S], F32, name="ef_t")
        lm_t = wp.tile([BS, S], F32, name="lm_t")
        la_t = wp.tile([BS, S], F32, name="la_t")
        da_t = wp.tile([BS, S], F32, name="da_t")
        la2 = wp.tile([BS, S], F32, name="la2")
        la2r = wp.tile([BS, S], F32, name="la2r")
        lb2 = wp.tile([BS, S], F32, name="lb2")
        lam = wp.tile([BS, S], F32, name="lam")
        m_t = wp.tile([BS, 1], F32, name="m_t")
        nm_t = wp.tile([BS, 1], F32, name="nm_t")
        e_t = wp.tile([BS, S], F32, name="e_t")
        z_t = wp.tile([BS, 1], F32, name="z_t")
        lnz_t = wp.tile([BS, 1], F32, name="lnz_t")
        racc = wp.tile([BS, 1], F32, name="racc")
        acc1 = wp.tile([BS, 1], F32, name="acc1")
        acc2 = wp.tile([BS, 1], F32, name="acc2")
        outt = wp.tile([BS, 1], F32, name="outt")

        def side_log(x32, out_lam):
            # exact log of f32 x: split exponent/mantissa so the Ln LUT only
            # sees [1,2); zero lanes forced to DEAD. The biased-exponent
            # -127*ln2 per side is folded into the final constant.
            v.tensor_scalar(out=as_t[:], in0=x32[:], scalar1=1.0, scalar2=TINY, op0=ALU.mult, op1=ALU.add)
            ai = as_t[:].bitcast(I32)
            v.tensor_scalar(out=ei_t[:], in0=ai, scalar1=23, scalar2=None, op0=ALU.logical_shift_right)
            v.tensor_scalar(out=mi_t[:], in0=ai, scalar1=0x007FFFFF, scalar2=0x3F800000, op0=ALU.bitwise_and, op1=ALU.bitwise_or)
            v.tensor_copy(out=ef_t[:], in_=ei_t[:])
            nc.scalar.activation(out=lm_t[:], in_=mi_t[:].bitcast(F32), func=ACTF.Ln)
            v.scalar_tensor_tensor(out=la_t[:], in0=ef_t[:], scalar=LN2, in1=lm_t[:], op0=ALU.mult, op1=ALU.add)
            v.tensor_scalar(out=da_t[:], in0=x32[:], scalar1=0.0, scalar2=None, op0=ALU.is_equal)
            v.scalar_tensor_tensor(out=out_lam[:], in0=da_t[:], scalar=DEAD, in1=la_t[:], op0=ALU.mult, op1=ALU.add)

        v.tensor_copy(out=ff32[:], in_=xfin[:, 0:S])
        side_log(ff32, la2)
        v.tensor_copy(out=bf32[:], in_=c_t[:, HW : HW + S])
        side_log(bf32, lb2)
        # un-reverse the fwd side so lanes line up with the bwd side
        v.tensor_copy(out=la2r[:, 0:S], in_=la2[:, S - 1 :: -1])
        v.tensor_tensor(out=lam[:], in0=la2r[:], in1=lb2[:], op=ALU.add)
        v.tensor_reduce(out=m_t[:], in_=lam[:], axis=AXX, op=ALU.max)
        v.tensor_scalar(out=nm_t[:], in0=m_t[:], scalar1=-1.0, scalar2=None, op0=ALU.mult)
        nc.scalar.activation(out=e_t[:], in_=lam[:], func=ACTF.Exp, bias=nm_t[:, 0:1], scale=1.0)
        v.tensor_reduce(out=z_t[:], in_=e_t[:], axis=AXX, op=ALU.add)
        nc.scalar.activation(out=lnz_t[:], in_=z_t[:], func=ACTF.Ln)
        v.tensor_copy(out=lgall[:], in_=lgi[:])
        v.tensor_reduce(out=racc[:], in_=lgall[:], axis=AXX, op=ALU.add)
        # constants: -2*RT for the host 2^110 injections, -2*127 for the two
        # side_log biased exponents.
        v.tensor_scalar(out=racc[:], in0=racc[:], scalar1=float(-2 * RT_LOG2 - 254), scalar2=None, op0=ALU.add)
        v.scalar_tensor_tensor(out=acc1[:], in0=racc[:], scalar=LN2, in1=m_t[:], op0=ALU.mult, op1=ALU.add)
        v.tensor_tensor(out=acc2[:], in0=acc1[:], in1=lnz_t[:], op=ALU.add)
        v.tensor_scalar(out=outt[:], in0=acc2[:], scalar1=-1.0, scalar2=None, op0=ALU.mult)
        nc.sync.dma_start(out=out_d, in_=outt[:])


def _build_program():
    nc = bacc.Bacc("TRN2", target_bir_lowering=False, debug=False)
    p_d = nc.dram_tensor("p", [BS, TH, W2], BF16, kind="ExternalInput").ap()
    pm_d = nc.dram_tensor("pm", [BS, TH, W2], BF16, kind="ExternalInput").ap()
    out_d = nc.dram_tensor("out", [BS, 1], F32, kind="ExternalOutput").ap()
    with tile.TileContext(nc) as tc:
        _emit(nc, tc, p_d, pm_d, out_d)
    nc.compile()
    return nc


_NC = None


def _get_nc():
    global _NC
    if _NC is None:
        _NC = _build_program()
    return _NC


def _prep_in_maps(y_pred, y_true, label_length):
    ext = np.full((B, S), BLANK, np.int32)
    ext[:, 1::2] = y_true.astype(np.int32)
    prev2 = np.concatenate([np.full((B, 2), BLANK, np.int32), ext[:, :-2]], axis=1)
    skip = ((ext != BLANK) & (ext != prev2)).astype(np.float32)
    skip2 = np.concatenate([skip[:, 2:], np.zeros((B, 2), np.float32)], axis=1)
    P = np.take_along_axis(
        np.ascontiguousarray(y_pred, dtype=np.float32), ext[:, None, :], axis=2
    )
    P += np.float32(EPS)
    L = label_length.reshape(B).astype(np.int64)
    i2 = np.clip(2 * L, 0, S - 1)
    i1 = np.maximum(i2 - 1, 0)
    # reachability mask: position s at time t is dead if it cannot reach i1
    # by t=T-1 (max +2 per step). Folded into P at zero device cost.
    s_idx = np.arange(S)[None, None, :]
    t_idx = np.arange(T)[None, :, None]
    alive = (s_idx + 2 * (T - 1 - t_idx)) >= i1[:, None, None]
    P *= alive.astype(np.float32)
    # host prescale: scale each (b,t) row to max ~1 (exact powers of two);
    # the exact correction Sum k_t * ln2 is added back on the host.
    k_t = np.round(np.log2(P.max(2))).astype(np.int32)
    P = P * np.exp2(-k_t.astype(np.float64))[:, :, None].astype(np.float32)
    kcorr = k_t.sum(1).astype(np.float64)
    sel = np.zeros((B, S), np.float32)
    sel[np.arange(B), i1] = 1.0
    sel[np.arange(B), i2] = 1.0
    SCALE = np.float32(2.0**RT_LOG2)
    # fused rows: left = s-reversed fwd probs, right = bwd probs (t falling)
    pfull = np.zeros((B, TH, W2), np.float32)
    pmfull = np.zeros((B, TH, W2), np.float32)
    pfull[:, 1:, 0:S] = P[:, 1:TH, ::-1]
    pfull[:, 1:, HW : HW + S] = P[:, T - 2 : T - 1 - TH : -1, :]
    skr = skip[:, ::-1]
    pmfull[:, 1:, 0:S] = pfull[:, 1:, 0:S] * skr[:, None, :]
    pmfull[:, 1:, HW : HW + S] = pfull[:, 1:, HW : HW + S] * skip2[:, None, :]
    # init row: [rev alpha_0 | sel*p_255] * 2^110
    pfull[:, 0, S - 2] = P[:, 0, 1] * SCALE
    pfull[:, 0, S - 1] = P[:, 0, 0] * SCALE
    pfull[:, 0, HW : HW + S] = sel * P[:, T - 1, :] * SCALE
    # PM slot 0 = seam mask [0 | skip2] for the final partial step
    pmfull[:, 0, HW : HW + S] = skip2
    pfull = pfull.astype(ml_dtypes.bfloat16)
    pmfull = pmfull.astype(ml_dtypes.bfloat16)
    in_maps = []
    for c in range(NCORES):
        sl = slice(c * BS, (c + 1) * BS)
        in_maps.append(
            {
                "p": np.ascontiguousarray(pfull[sl]),
                "pm": np.ascontiguousarray(pmfull[sl]),
            }
        )
    return in_maps, kcorr


def _run_device(in_maps, **kwargs):
    nc = _get_nc()
    return run_bass_kernel_spmd(nc, in_maps, core_ids=list(range(NCORES)), **kwargs)


def _ctc_numpy(y_pred, y_true, input_length, label_length):
    """Generality safety net (log domain, mirrors the reference exactly)."""
    b, t_max, c = y_pred.shape
    u = y_true.shape[1]
    s = 2 * u + 1
    blank = c - 1
    neg = np.float32(-1e30)
    logp = np.log(y_pred.astype(np.float32) + np.float32(EPS))
    ext = np.full((b, s), blank, np.int32)
    ext[:, 1::2] = y_true.astype(np.int32)
    prev2 = np.concatenate([np.full((b, 2), blank, np.int32), ext[:, :-2]], axis=1)
    can_skip = (ext != blank) & (ext != prev2)
    lp_ext = np.take_along_axis(logp, ext[:, None, :], axis=2)
    alpha = np.full((b, s), neg, np.float32)
    alpha[:, 0] = lp_ext[:, 0, 0]
    alpha[:, 1] = lp_ext[:, 0, 1]
    inp_len = input_length.reshape(b)

    def lse(stack):
        m = np.max(stack, axis=0)
        return m + np.log(np.sum(np.exp(stack - m), axis=0))

    for t in range(1, t_max):
        a1 = np.concatenate([np.full((b, 1), neg, np.float32), alpha[:, :-1]], axis=1)
        a2 = np.concatenate([np.full((b, 2), neg, np.float32), alpha[:, :-2]], axis=1)
        a2 = np.where(can_skip, a2, neg)
        new = lse(np.stack([alpha, a1, a2], 0)).astype(np.float32) + lp_ext[:, t, :]
        alpha = np.where((t < inp_len)[:, None], new, alpha)
    L = label_length.reshape(b).astype(np.int64)
    i2 = np.clip(2 * L, 0, s - 1)
    i1 = np.maximum(i2 - 1, 0)
    a_last = np.stack([alpha[np.arange(b), i1], alpha[np.arange(b), i2]], axis=1)
    ll = np.where(L > 0, lse(a_last.T).astype(np.float32), alpha[:, 0])
    return (-ll[:, None]).astype(np.float32)


def kernel(y_pred, y_true, input_length, label_length):
    y_pred = np.asarray(y_pred)
    y_true = np.asarray(y_true)
    input_length = np.asarray(input_length)
    label_length = np.asarray(label_length)
    if y_pred.shape != (B, T, C) or y_true.shape != (B, U) or not np.all(
        input_length.reshape(-1) == T
    ) or np.any(label_length.reshape(-1) <= 0):
        return _ctc_numpy(y_pred, y_true, input_length, label_length)
    in_maps, kcorr = _prep_in_maps(y_pred, y_true, label_length)
    res = _run_device(in_maps)
    out = np.concatenate([r["out"] for r in res.results], axis=0)
    out = out - (LN2 * kcorr)[:, None].astype(np.float32)
    return np.ascontiguousarray(out, dtype=np.float32)
